# revision 31
# baseline (speedup 1.0000x reference)
"""CPC loss kernel for Trainium2 (8 NeuronCores, SPMD data-parallel over batch N).

Math (per batch element n, handled by core n):
  Az[t]   = W @ latent[n, t]            (K*C = 3072 outputs per position)
  scores[t, k, m] = phi[s_{t,m}] . Az[t, k]   (M=128 gathered negatives)
  num[t, k]       = latent[n, 1+t+k] . Az[t, k]
  loss = mean over (n, t<500, k) of log(sum_m exp(scores) + exp(num)) - num

Device strategy per core:
  - bf16 transpose-mode dma_gather pulls the 128 negatives per position
    directly in [c, m] layout from a replicated bf16 latent table; 7 positions
    (896 indices) per gather (SWDGE ring caps one gather at ~1008 indices),
    round-robined over 4 SWDGE queues so desc-gen overlaps transfers.
  - Positives need no gather: latent[n, 1+t+k] for k=0..11 are 12 contiguous
    columns of the transposed own-latent tile.
  - AzT is computed once via PE and stored bf16 in SBUF so each (t, c_half)
    exposes a contiguous 32-col weight slab (12 real k + 20 zero pad cols so
    the pad output partitions produce zero scores).
  - Per position, 4 accumulating matmuls (2 c-halves x {positives, negatives})
    with 4-way column tiling produce [4*32, 140] score tiles in PSUM;
    a DVE tensor_tensor_reduce extracts the positive diagonal into num_all,
    one ACT exp(x-50) with accum_out yields sum_m exp(scores-50) in tot_all,
    and a per-megatile exp+add folds in exp(num-50).
  - Final: ln(tot*2^-32), subtract num, masked partition-sum via 1-col matmul.
Host: loss = sum(partials)/48000 + 50 + 32*ln(2).
"""

import sys, os

_ABL = os.environ.get("KABL", "")

for _p in ("/opt/trn_rl_repo", "/root/.axon_site/_ro/trn_rl_repo"):
    if _p not in sys.path:
        sys.path.append(_p)

import numpy as np
import ml_dtypes

import concourse.bass as bass
import concourse.bacc as bacc
import concourse.mybir as mybir
from concourse.tile import TileContext, add_dep_helper
from concourse import library_config

BF16 = ml_dtypes.bfloat16

N, T, C, K, M = 8, 512, 256, 12, 128
Tp = T - K  # 500 real positions
TPAD = 512  # padded position count (32 PSUM megatiles of 16)
IPP = M + K  # 140 score columns per position (12 positives, 128 negatives)
SHIFT = 50.0  # fixed logsumexp shift; |scores| << SHIFT + 88 so exp never overflows
DENOM = N * Tp * K  # 48000

PPG = 7  # positions per gather: 7*128 = 896 indices (ring caps at 1008)
NG_FULL = Tp // PPG  # 71 full gathers
LAST_P = Tp - NG_FULL * PPG  # 3 positions in the last gather
NGATH = NG_FULL + 1  # 72
IDXC = NG_FULL * (PPG * M // 16) + LAST_P * M // 16  # idx cols: 71*56 + 24 = 4000
NQ = int(os.environ.get("KNQ", "4"))  # SWDGE queues


def build_bass():
    nc = bacc.Bacc(
        "TRN2",
        target_bir_lowering=False,
        debug=False,
        enable_asserts=False,
        num_swdge_queues=NQ,
    )
    dt = mybir.dt

    lat_all = nc.dram_tensor("lat_all", [N * T, C], dt.bfloat16, kind="ExternalInput").ap()
    latT = nc.dram_tensor("latT", [128, 2, T], dt.bfloat16, kind="ExternalInput").ap()
    wT = nc.dram_tensor("wT", [128, 2, K * C], dt.bfloat16, kind="ExternalInput").ap()
    idx = nc.dram_tensor("idx", [128, IDXC], dt.int16, kind="ExternalInput").ap()
    pmask = nc.dram_tensor("pmask", [128, 1], dt.float32, kind="ExternalInput").ap()
    maskI = nc.dram_tensor("maskI", [128, K], dt.float32, kind="ExternalInput").ap()
    out = nc.dram_tensor("out", [1, 1], dt.float32, kind="ExternalOutput").ap()

    with TileContext(nc) as tc:
        nc.gpsimd.load_library(library_config.mlp)
        with (
            tc.tile_pool(name="const", bufs=1) as cp,
            tc.tile_pool(name="gat", bufs=10) as gp,
            tc.tile_pool(name="scr", bufs=4) as sp,
            tc.tile_pool(name="acc", bufs=1) as ap_,
        ):
            # --- constant / weight loads -------------------------------------
            latT_t = cp.tile([128, 2, T], dt.bfloat16)
            nc.sync.dma_start(latT_t[:], latT[:])
            wT_t = cp.tile([128, 2, K * C], dt.bfloat16)
            nc.sync.dma_start(wT_t[:], wT[:])
            pmask_t = cp.tile([128, 1], dt.float32)
            nc.sync.dma_start(pmask_t[:], pmask[:])
            maskI_t = cp.tile([128, K], dt.float32)
            nc.sync.dma_start(maskI_t[:], maskI[:])
            negshift = cp.tile([128, 1], dt.float32)
            nc.vector.memset(negshift[:], -SHIFT)
            idx_t = cp.tile([128, IDXC], dt.int16)
            nc.sync.dma_start(idx_t[:], idx[:])

            # AzT store: azsb[p, (2t+h)*32 + k] = Az[k, h*128+p, t] (bf16).
            azsb = ap_.tile([128, TPAD * 64], dt.bfloat16)
            nc.vector.memset(azsb[:], 0.0)
            azsb3 = azsb.rearrange("p (t x) -> p t x", x=64)

            tot_all = ap_.tile([128, TPAD // 4], dt.float32)
            num_all = ap_.tile([128, TPAD // 4], dt.float32)

            # --- Az phase: AzT[kc, t] = sum_c' W[kc, c'] latent[n, t, c'] ----
            with tc.tile_pool(name="az_ps", bufs=2, space="PSUM") as azps:
                for b in range(2 * K):  # kc tile: kc = b*128 + p
                    k_, h_ = b // 2, b % 2
                    pa = azps.tile([128, T], dt.float32, name="pa")
                    for hp in range(2):  # contraction half
                        nc.tensor.matmul(
                            pa[:, :],
                            lhsT=wT_t[:, hp, b * 128 : (b + 1) * 128],
                            rhs=latT_t[:, hp, :],
                            start=(hp == 0),
                            stop=(hp == 1),
                        )
                    nc.scalar.copy(out=azsb3[:, :, 32 * h_ + k_], in_=pa[:, :])

            # --- negative gathers (t < 500 only) -----------------------------
            nidx_regs = {
                PPG * M: nc.gpsimd.to_reg(PPG * M),
                LAST_P * M: nc.gpsimd.to_reg(LAST_P * M),
            }
            ng_tiles = []
            prev_gather = None
            _ngath = int(os.environ.get("KGATH", NGATH))
            for g in range(NGATH):
                if g >= _ngath:
                    ng_tiles.append(None)
                    continue
                npos = PPG if g < NG_FULL else LAST_P
                nidx = npos * M
                if g < NG_FULL:
                    g_t = gp.tile([128, 2, PPG * M], dt.bfloat16, tag="ng", name="ng")
                else:
                    g_t = gp.tile([128, 2, nidx], dt.bfloat16, tag="ng_last", name="ngl")
                ics = g * (PPG * M // 16)
                gi = nc.gpsimd.dma_gather(
                    g_t[:],
                    lat_all[:],
                    idx_t[:, ics : ics + nidx // 16],
                    nidx,
                    nidx_regs[nidx],
                    C,
                    transpose=True,
                    queue_num=g % NQ,
                )
                # Pin gather scheduling order: the Tile DMASW-lane round-robin
                # must stay in lockstep with queue_num (a DMASW sem is locked
                # to one SWDGE queue).
                if prev_gather is not None:
                    add_dep_helper(gi.ins, prev_gather.ins, sync=False, reason="gather order")
                prev_gather = gi
                ng_tiles.append(g_t)

            # --- score megatiles ---------------------------------------------
            _nmega = int(os.environ.get("KLIM", TPAD // 16))
            with tc.tile_pool(name="sc_ps", bufs=2, space="PSUM") as scps:
                for mega in range(_nmega):
                    P = scps.tile([128, 4, 512], dt.float32, name="P")
                    for s in range(4):  # bank = one 4-position score tile
                        tile_idx = mega * 4 + s
                        for q in range(4):  # column group
                            t = tile_idx * 4 + q
                            if t < Tp:
                                g, pl = t // PPG, t % PPG
                                neg_rhs = lambda h, g=g, pl=pl: ng_tiles[g][
                                    :, h, M * pl : M * (pl + 1)
                                ]
                            else:  # dummy position: zero weights, any rhs
                                neg_rhs = lambda h: latT_t[:, h, 0:M]
                            pt = min(1 + t, T - K)  # positive cols (clamped for pads)
                            # one accumulation group per (q, s): the bank is a
                            # single has_written zero-region; first touch
                            # overwrites, second (other c-half) accumulates
                            for h in range(2):
                                if "nomm" in _ABL:
                                    break
                                slab = azsb[:, (2 * t + h) * 32 : (2 * t + h) * 32 + 32]
                                nc.tensor.matmul(
                                    P[32 * q : 32 * q + 32, s, 0:K],
                                    lhsT=slab,
                                    rhs=latT_t[:, h, pt : pt + K],
                                    start=(h == 0),
                                    stop=False,
                                    tile_position=(0, 32 * q),
                                )
                                nc.tensor.matmul(
                                    P[32 * q : 32 * q + 32, s, K:IPP],
                                    lhsT=slab,
                                    rhs=neg_rhs(h),
                                    start=False,
                                    stop=(h == 1),
                                    tile_position=(0, 32 * q),
                                )
                        if "notail" in _ABL:
                            continue
                        # num[t,k] -> num_all (diagonal of the positive block)
                        scr = sp.tile([128, K], dt.float32, tag="ttr", name="ttr_o")
                        nc.vector.tensor_mul(scr[:], P[:, s, 0:K], maskI_t[:])
                        nc.vector.tensor_reduce(
                            num_all[:, tile_idx : tile_idx + 1],
                            scr[:],
                            axis=mybir.AxisListType.X,
                            op=mybir.AluOpType.add,
                        )
                        # tot[t,k] = sum_m exp(score-50)
                        nc.scalar.activation(
                            out=sp.tile([128, M], dt.float32, tag="exp", name="exp_o"),
                            in_=P[:, s, K:IPP],
                            func=mybir.ActivationFunctionType.Exp,
                            bias=negshift[:],
                            scale=1.0,
                            accum_out=tot_all[:, tile_idx : tile_idx + 1],
                        )
                    if "notail" in _ABL:
                        continue
                    # fold in the positive term: tot += exp(num - 50)
                    en_t = sp.tile([128, 4], dt.float32, tag="en", name="en_t")
                    nc.scalar.activation(
                        out=en_t[:],
                        in_=num_all[:, mega * 4 : mega * 4 + 4],
                        func=mybir.ActivationFunctionType.Exp,
                        bias=negshift[:],
                        scale=1.0,
                    )
                    nc.vector.tensor_add(
                        tot_all[:, mega * 4 : mega * 4 + 4],
                        tot_all[:, mega * 4 : mega * 4 + 4],
                        en_t[:],
                    )

            # --- final reduction --------------------------------------------
            if "nofin" in _ABL:
                dummy = ap_.tile([1, 1], dt.float32)
                nc.vector.memset(dummy[:], 0.0)
                nc.sync.dma_start(out[:], dummy[:])
            else:
                NV = Tp // 4  # 125 valid score tiles
                # ln(tot * 2^-32) keeps the ACT-ln input within its 2^64 valid
                # range for extreme scores; +32*ln2 is restored on the host.
                Lt = ap_.tile([128, NV], dt.float32)
                nc.scalar.activation(
                    out=Lt[:],
                    in_=tot_all[:, :NV],
                    func=mybir.ActivationFunctionType.Ln,
                    scale=float(2.0**-32),
                )
                Dt = ap_.tile([128, NV], dt.float32)
                rs = ap_.tile([128, 1], dt.float32)
                nc.vector.tensor_sub(Dt[:], Lt[:], num_all[:, :NV])
                nc.vector.tensor_reduce(
                    rs[:],
                    Dt[:],
                    axis=mybir.AxisListType.X,
                    op=mybir.AluOpType.add,
                )
                with tc.tile_pool(name="f_ps", bufs=1, space="PSUM") as fps:
                    psf = fps.tile([1, 1], dt.float32)
                    nc.tensor.matmul(psf[:], lhsT=rs[:], rhs=pmask_t[:])
                    outsb = ap_.tile([1, 1], dt.float32)
                    nc.scalar.copy(out=outsb[:], in_=psf[:])
                    nc.sync.dma_start(out[:], outsb[:])

    nc.compile()
    return nc


def prep_inputs(latent, W, samps):
    """Host-side sharding + layout marshalling. Returns per-core input maps."""
    latent = np.asarray(latent, dtype=np.float32)
    W = np.asarray(W, dtype=np.float32)
    samps = np.asarray(samps).astype(np.int64).reshape(N, Tp, M)

    lat_all = latent.reshape(N * T, C).astype(BF16)
    wT = np.ascontiguousarray(
        W.T.astype(BF16).reshape(2, 128, K * C).transpose(1, 0, 2)
    )
    pmask = ((np.arange(128) % 32) < K).astype(np.float32).reshape(128, 1)
    maskI = (np.arange(K)[None, :] == (np.arange(128) % 32)[:, None]).astype(np.float32)

    in_maps = []
    for n in range(N):
        latT = np.ascontiguousarray(
            latent[n].T.astype(BF16).reshape(2, 128, T).transpose(1, 0, 2)
        )
        # negative gather indices, wrapped: idx[p, g*56 + s] = flat_g[s*16 + p%16]
        flat = samps[n].reshape(Tp * M).astype(np.int16)  # position-major
        wrapped = flat.reshape(IDXC, 16).T  # [16, IDXC]
        idx = np.ascontiguousarray(np.tile(wrapped, (8, 1)))
        in_maps.append(
            {
                "lat_all": lat_all,
                "latT": latT,
                "wT": wT,
                "idx": idx,
                "pmask": pmask,
                "maskI": maskI,
            }
        )
    return in_maps


_NC_CACHE = None


def kernel(latent, W, samps):
    global _NC_CACHE
    from concourse import bass_utils

    if _NC_CACHE is None:
        _NC_CACHE = build_bass()
    nc = _NC_CACHE
    in_maps = prep_inputs(latent, W, samps)
    res = bass_utils.run_bass_kernel_spmd(nc, in_maps, core_ids=list(range(N)))
    partial = sum(float(r["out"][0, 0]) for r in res.results)
    import math

    return np.float32(partial / DENOM + SHIFT + 32.0 * math.log(2.0))


# revision 41
# speedup vs baseline: 1.2672x; 1.2672x over previous
"""CPC loss kernel for Trainium2 (8 NeuronCores, SPMD data-parallel over batch N).

Math (per batch element n, handled by core n):
  Az[t]   = W @ latent[n, t]            (K*C = 3072 outputs per position)
  scores[t, k, m] = phi[s_{t,m}] . Az[t, k]   (M=128 gathered negatives)
  num[t, k]       = latent[n, 1+t+k] . Az[t, k]
  loss = mean over (n, t<500, k) of log(sum_m exp(scores) + exp(num)) - num

Device strategy per core:
  - bf16 transpose-mode dma_gather pulls the 128 negatives per position
    directly in [c, m] layout from a replicated bf16 latent table; 7 positions
    (896 indices) per gather (SWDGE ring caps one gather at ~1008 indices),
    round-robined over 4 SWDGE queues so desc-gen overlaps transfers.
  - Positives need no gather: latent[n, 1+t+k] for k=0..11 are 12 contiguous
    columns of the transposed own-latent tile.
  - AzT is computed once via PE and stored bf16 in SBUF so each (t, c_half)
    exposes a contiguous 32-col weight slab (12 real k + 20 zero pad cols so
    the pad output partitions produce zero scores).
  - Per position, 4 accumulating matmuls (2 c-halves x {positives, negatives})
    with 4-way column tiling produce [4*32, 140] score tiles in PSUM;
    a DVE tensor_tensor_reduce extracts the positive diagonal into num_all,
    one ACT exp(x-50) with accum_out yields sum_m exp(scores-50) in tot_all,
    and a per-megatile exp+add folds in exp(num-50).
  - Final: ln(tot*2^-32), subtract num, masked partition-sum via 1-col matmul.
Host: loss = sum(partials)/48000 + 50 + 32*ln(2).
"""

import sys, os

_ABL = os.environ.get("KABL", "")

for _p in ("/opt/trn_rl_repo", "/root/.axon_site/_ro/trn_rl_repo"):
    if _p not in sys.path:
        sys.path.append(_p)

import numpy as np
import ml_dtypes

import concourse.bass as bass
import concourse.bacc as bacc
import concourse.mybir as mybir
from concourse.tile import TileContext, add_dep_helper
from concourse import library_config

BF16 = ml_dtypes.bfloat16

N, T, C, K, M = 8, 512, 256, 12, 128
Tp = T - K  # 500 real positions
TPAD = 512  # padded position count (32 PSUM megatiles of 16)
PB = 15  # positive-block cols per bank: shared rhs window covers 4 positions
IPP = M + K  # legacy name (unused in scores layout)
SHIFT = 50.0  # fixed logsumexp shift; |scores| << SHIFT + 88 so exp never overflows
DENOM = N * Tp * K  # 48000

PPG = 7  # positions per gather: 7*128 = 896 indices (ring caps at 1008)
NG_FULL = Tp // PPG  # 71 full gathers
LAST_P = Tp - NG_FULL * PPG  # 3 positions in the last gather
NGATH = NG_FULL + 1  # 72
IDXC = NG_FULL * (PPG * M // 16) + LAST_P * M // 16  # idx cols: 71*56 + 24 = 4000
NQ = int(os.environ.get("KNQ", "4"))  # SWDGE queues


def build_bass():
    nc = bacc.Bacc(
        "TRN2",
        target_bir_lowering=False,
        debug=False,
        enable_asserts=False,
        num_swdge_queues=NQ,
    )
    dt = mybir.dt

    lat_all = nc.dram_tensor("lat_all", [N * T, C], dt.bfloat16, kind="ExternalInput").ap()
    latT = nc.dram_tensor("latT", [128, 2, T], dt.bfloat16, kind="ExternalInput").ap()
    wT = nc.dram_tensor("wT", [128, 2, K * C], dt.bfloat16, kind="ExternalInput").ap()
    idx = nc.dram_tensor("idx", [128, IDXC], dt.int16, kind="ExternalInput").ap()
    pmask = nc.dram_tensor("pmask", [128, 1], dt.float32, kind="ExternalInput").ap()
    maskI = nc.dram_tensor("maskI", [128, PB], dt.float32, kind="ExternalInput").ap()
    out = nc.dram_tensor("out", [1, 1], dt.float32, kind="ExternalOutput").ap()

    with TileContext(nc) as tc:
        nc.gpsimd.load_library(library_config.mlp)
        with (
            tc.tile_pool(name="const", bufs=1) as cp,
            tc.tile_pool(name="gat", bufs=14) as gp,
            tc.tile_pool(name="scr", bufs=4) as sp,
            tc.tile_pool(name="acc", bufs=1) as ap_,
        ):
            # --- constant / weight loads -------------------------------------
            latT_t = cp.tile([128, 2, T], dt.bfloat16)
            nc.sync.dma_start(latT_t[:], latT[:])
            wT_t = cp.tile([128, 2, K * C], dt.bfloat16)
            nc.sync.dma_start(wT_t[:], wT[:])
            pmask_t = cp.tile([128, 1], dt.float32)
            nc.sync.dma_start(pmask_t[:], pmask[:])
            maskI_t = cp.tile([128, PB], dt.float32)
            nc.sync.dma_start(maskI_t[:], maskI[:])
            negshift = cp.tile([128, 1], dt.float32)
            nc.vector.memset(negshift[:], -SHIFT)
            idx_t = cp.tile([128, IDXC], dt.int16)
            nc.sync.dma_start(idx_t[:], idx[:])

            # AzT store, tile-major: azsb[p, u*256 + h*128 + q*32 + k]
            # = Az[k, h*128+p, t=4u+q] (bf16). Each (tile u, c-half h) owns a
            # contiguous 128-col slab of 4 position sub-slabs (12 real k + 20
            # zero pad cols so pad output partitions produce zero scores).
            azsb = ap_.tile([128, TPAD * 64], dt.bfloat16)
            nc.vector.memset(azsb[:], 0.0)
            azsb6 = azsb.rearrange("p (u hh q j) -> p u hh q j", hh=2, q=4, j=32)

            tot_all = ap_.tile([128, TPAD // 4], dt.float32)
            num_all = ap_.tile([128, TPAD // 4], dt.float32)

            # --- Az phase: AzT[kc, t] = sum_c' W[kc, c'] latent[n, t, c'] ----
            with tc.tile_pool(name="az_ps", bufs=2, space="PSUM") as azps:
                for b in range(2 * K):  # kc tile: kc = b*128 + p
                    k_, h_ = b // 2, b % 2
                    pa = azps.tile([128, T], dt.float32, name="pa")
                    for hp in range(2):  # contraction half
                        nc.tensor.matmul(
                            pa[:, :],
                            lhsT=wT_t[:, hp, b * 128 : (b + 1) * 128],
                            rhs=latT_t[:, hp, :],
                            start=(hp == 0),
                            stop=(hp == 1),
                        )
                    nc.scalar.copy(out=azsb6[:, :, h_, :, k_], in_=pa[:, :])

            # --- negative gathers (t < 500 only) -----------------------------
            nidx_regs = {
                PPG * M: nc.gpsimd.to_reg(PPG * M),
                LAST_P * M: nc.gpsimd.to_reg(LAST_P * M),
            }
            ng_tiles = []
            prev_gather = None
            _ngath = int(os.environ.get("KGATH", NGATH))
            for g in range(NGATH):
                if g >= _ngath:
                    ng_tiles.append(None)
                    continue
                npos = PPG if g < NG_FULL else LAST_P
                nidx = npos * M
                if g < NG_FULL:
                    g_t = gp.tile([128, 2, PPG * M], dt.bfloat16, tag="ng", name="ng")
                else:
                    g_t = gp.tile([128, 2, nidx], dt.bfloat16, tag="ng_last", name="ngl")
                ics = g * (PPG * M // 16)
                gi = nc.gpsimd.dma_gather(
                    g_t[:],
                    lat_all[:],
                    idx_t[:, ics : ics + nidx // 16],
                    nidx,
                    nidx_regs[nidx],
                    C,
                    transpose=True,
                    queue_num=g % NQ,
                )
                # Pin gather scheduling order: the Tile DMASW-lane round-robin
                # must stay in lockstep with queue_num (a DMASW sem is locked
                # to one SWDGE queue).
                if prev_gather is not None:
                    add_dep_helper(gi.ins, prev_gather.ins, sync=False, reason="gather order")
                prev_gather = gi
                ng_tiles.append(g_t)

            # --- score megatiles ---------------------------------------------
            _nmega = int(os.environ.get("KLIM", TPAD // 16))
            with tc.tile_pool(name="sc_ps", bufs=2, space="PSUM") as scps:
                for mega in range(_nmega):
                    P = scps.tile([128, 4, 512], dt.float32, name="P")
                    exp_i = None
                    for s in range(4):  # bank = one 4-position score tile
                        tile_idx = mega * 4 + s
                        t0 = tile_idx * 4
                        # positive cols: one shared 15-col window for all 4
                        # positions of the tile (diag shifts by col group q)
                        pt = min(1 + t0, T - PB)  # clamp pads in-bounds
                        if "nomm" in _ABL:
                            continue
                        # batched positives: strided 128-col lhsT covers the 4
                        # positions' weight slabs -> one group over all rows
                        pos_close = None
                        for h in range(2):
                            slab4 = azsb[:, tile_idx * 256 + h * 128 : tile_idx * 256 + (h + 1) * 128]
                            pos_close = nc.tensor.matmul(
                                P[:, s, 0:PB],
                                lhsT=slab4,
                                rhs=latT_t[:, h, pt : pt + PB],
                                start=(h == 0),
                                stop=(h == 1),
                            )
                        for q in range(4):  # column group: own 2-MM group
                            t = t0 + q
                            for h in range(2):
                                if t < Tp:
                                    g, pl = t // PPG, t % PPG
                                    nrhs = ng_tiles[g][:, h, M * pl : M * (pl + 1)]
                                else:  # dummy position: zero weights, any rhs
                                    nrhs = latT_t[:, h, 0:M]
                                slab = azsb[
                                    :,
                                    tile_idx * 256 + h * 128 + q * 32 : tile_idx * 256 + h * 128 + q * 32 + 32,
                                ]
                                mm = nc.tensor.matmul(
                                    P[32 * q : 32 * q + 32, s, PB : PB + M],
                                    lhsT=slab,
                                    rhs=nrhs,
                                    start=(h == 0),
                                    stop=(h == 1),
                                    tile_position=(0, 32 * q),
                                )
                                if h == 0:
                                    # the neg group's start clears the bank's
                                    # has_written rows: order it after the
                                    # positive group closes
                                    add_dep_helper(mm.ins, pos_close.ins, sync=False, reason="pos first")
                    if "notail" in _ABL:
                        continue
                    # tot[t,k] = sum_m exp(score-50): one exp over all 4 banks
                    E4 = sp.tile([128, 4, M], dt.float32, tag="exp", name="exp_o")
                    exp_i = nc.scalar.activation(
                        out=E4[:],
                        in_=P[:, :, PB : PB + M],
                        func=mybir.ActivationFunctionType.Exp,
                        bias=negshift[:],
                        scale=1.0,
                    )
                    nc.vector.tensor_reduce(
                        tot_all[:, mega * 4 : mega * 4 + 4],
                        E4[:],
                        axis=mybir.AxisListType.X,
                        op=mybir.AluOpType.add,
                    )
                    # num[t,k] -> num_all (shifted diagonal of the pos block),
                    # after the exp so every bank group is closed
                    for s in range(4):
                        tile_idx = mega * 4 + s
                        scr = sp.tile([128, PB], dt.float32, tag="ttr", name="ttr_o")
                        mul_i = nc.vector.tensor_mul(scr[:], P[:, s, 0:PB], maskI_t[:])
                        add_dep_helper(mul_i.ins, exp_i.ins, sync=True, reason="groups closed")
                        nc.vector.tensor_reduce(
                            num_all[:, tile_idx : tile_idx + 1],
                            scr[:],
                            axis=mybir.AxisListType.X,
                            op=mybir.AluOpType.add,
                        )
                    # fold in the positive term: tot += exp(num - 50)
                    en_t = sp.tile([128, 4], dt.float32, tag="en", name="en_t")
                    nc.scalar.activation(
                        out=en_t[:],
                        in_=num_all[:, mega * 4 : mega * 4 + 4],
                        func=mybir.ActivationFunctionType.Exp,
                        bias=negshift[:],
                        scale=1.0,
                    )
                    nc.vector.tensor_add(
                        tot_all[:, mega * 4 : mega * 4 + 4],
                        tot_all[:, mega * 4 : mega * 4 + 4],
                        en_t[:],
                    )

            # --- final reduction --------------------------------------------
            if "nofin" in _ABL:
                dummy = ap_.tile([1, 1], dt.float32)
                nc.vector.memset(dummy[:], 0.0)
                nc.sync.dma_start(out[:], dummy[:])
            else:
                NV = Tp // 4  # 125 valid score tiles
                # ln(tot * 2^-32) keeps the ACT-ln input within its 2^64 valid
                # range for extreme scores; +32*ln2 is restored on the host.
                Lt = ap_.tile([128, NV], dt.float32)
                nc.scalar.activation(
                    out=Lt[:],
                    in_=tot_all[:, :NV],
                    func=mybir.ActivationFunctionType.Ln,
                    scale=float(2.0**-32),
                )
                Dt = ap_.tile([128, NV], dt.float32)
                rs = ap_.tile([128, 1], dt.float32)
                nc.vector.tensor_sub(Dt[:], Lt[:], num_all[:, :NV])
                nc.vector.tensor_reduce(
                    rs[:],
                    Dt[:],
                    axis=mybir.AxisListType.X,
                    op=mybir.AluOpType.add,
                )
                with tc.tile_pool(name="f_ps", bufs=1, space="PSUM") as fps:
                    psf = fps.tile([1, 1], dt.float32)
                    nc.tensor.matmul(psf[:], lhsT=rs[:], rhs=pmask_t[:])
                    outsb = ap_.tile([1, 1], dt.float32)
                    nc.scalar.copy(out=outsb[:], in_=psf[:])
                    nc.sync.dma_start(out[:], outsb[:])

    nc.compile()
    return nc


def prep_inputs(latent, W, samps):
    """Host-side sharding + layout marshalling. Returns per-core input maps."""
    latent = np.asarray(latent, dtype=np.float32)
    W = np.asarray(W, dtype=np.float32)
    samps = np.asarray(samps).astype(np.int64).reshape(N, Tp, M)

    lat_all = latent.reshape(N * T, C).astype(BF16)
    wT = np.ascontiguousarray(
        W.T.astype(BF16).reshape(2, 128, K * C).transpose(1, 0, 2)
    )
    pmask = ((np.arange(128) % 32) < K).astype(np.float32).reshape(128, 1)
    q_arr, k_arr = np.arange(128) // 32, np.arange(128) % 32
    maskI = (
        (np.arange(15)[None, :] == (q_arr + k_arr)[:, None]) & (k_arr < K)[:, None]
    ).astype(np.float32)

    in_maps = []
    for n in range(N):
        latT = np.ascontiguousarray(
            latent[n].T.astype(BF16).reshape(2, 128, T).transpose(1, 0, 2)
        )
        # negative gather indices, wrapped: idx[p, g*56 + s] = flat_g[s*16 + p%16]
        flat = samps[n].reshape(Tp * M).astype(np.int16)  # position-major
        wrapped = flat.reshape(IDXC, 16).T  # [16, IDXC]
        idx = np.ascontiguousarray(np.tile(wrapped, (8, 1)))
        in_maps.append(
            {
                "lat_all": lat_all,
                "latT": latT,
                "wT": wT,
                "idx": idx,
                "pmask": pmask,
                "maskI": maskI,
            }
        )
    return in_maps


_NC_CACHE = None


def kernel(latent, W, samps):
    global _NC_CACHE
    from concourse import bass_utils

    if _NC_CACHE is None:
        _NC_CACHE = build_bass()
    nc = _NC_CACHE
    in_maps = prep_inputs(latent, W, samps)
    res = bass_utils.run_bass_kernel_spmd(nc, in_maps, core_ids=list(range(N)))
    partial = sum(float(r["out"][0, 0]) for r in res.results)
    import math

    return np.float32(partial / DENOM + SHIFT + 32.0 * math.log(2.0))


# revision 42
# speedup vs baseline: 1.4361x; 1.1333x over previous
"""CPC loss kernel for Trainium2 (8 NeuronCores, SPMD data-parallel over batch N).

Math (per batch element n, handled by core n):
  Az[t]   = W @ latent[n, t]            (K*C = 3072 outputs per position)
  scores[t, k, m] = phi[s_{t,m}] . Az[t, k]   (M=128 gathered negatives)
  num[t, k]       = latent[n, 1+t+k] . Az[t, k]
  loss = mean over (n, t<500, k) of log(sum_m exp(scores) + exp(num)) - num

Device strategy per core:
  - bf16 transpose-mode dma_gather pulls the 128 negatives per position
    directly in [c, m] layout from a replicated bf16 latent table; 7 positions
    (896 indices) per gather (SWDGE ring caps one gather at ~1008 indices),
    round-robined over 4 SWDGE queues so desc-gen overlaps transfers.
  - Positives need no gather: latent[n, 1+t+k] for k=0..11 are 12 contiguous
    columns of the transposed own-latent tile.
  - AzT is computed once via PE and stored bf16 in SBUF so each (t, c_half)
    exposes a contiguous 32-col weight slab (12 real k + 20 zero pad cols so
    the pad output partitions produce zero scores).
  - Per position, 4 accumulating matmuls (2 c-halves x {positives, negatives})
    with 4-way column tiling produce [4*32, 140] score tiles in PSUM;
    a DVE tensor_tensor_reduce extracts the positive diagonal into num_all,
    one ACT exp(x-50) with accum_out yields sum_m exp(scores-50) in tot_all,
    and a per-megatile exp+add folds in exp(num-50).
  - Final: ln(tot*2^-32), subtract num, masked partition-sum via 1-col matmul.
Host: loss = sum(partials)/48000 + 50 + 32*ln(2).
"""

import sys, os

_ABL = ""

for _p in ("/opt/trn_rl_repo", "/root/.axon_site/_ro/trn_rl_repo"):
    if _p not in sys.path:
        sys.path.append(_p)

import numpy as np
import ml_dtypes

import concourse.bass as bass
import concourse.bacc as bacc
import concourse.mybir as mybir
from concourse.tile import TileContext, add_dep_helper
from concourse import library_config

BF16 = ml_dtypes.bfloat16

N, T, C, K, M = 8, 512, 256, 12, 128
Tp = T - K  # 500 real positions
TPAD = 512  # padded position count (32 PSUM megatiles of 16)
PB = 15  # positive-block cols per bank: shared rhs window covers 4 positions
IPP = M + K  # legacy name (unused in scores layout)
SHIFT = 50.0  # fixed logsumexp shift; |scores| << SHIFT + 88 so exp never overflows
DENOM = N * Tp * K  # 48000

PPG = 7  # positions per gather: 7*128 = 896 indices (ring caps at 1008)
NG_FULL = Tp // PPG  # 71 full gathers
LAST_P = Tp - NG_FULL * PPG  # 3 positions in the last gather
NGATH = NG_FULL + 1  # 72
IDXC = NG_FULL * (PPG * M // 16) + LAST_P * M // 16  # idx cols: 71*56 + 24 = 4000
NQ = 4  # SWDGE queues


def build_bass():
    nc = bacc.Bacc(
        "TRN2",
        target_bir_lowering=False,
        debug=False,
        enable_asserts=False,
        num_swdge_queues=NQ,
    )
    dt = mybir.dt

    lat_all = nc.dram_tensor("lat_all", [N * T, C], dt.bfloat16, kind="ExternalInput").ap()
    latT = nc.dram_tensor("latT", [128, 2, T], dt.bfloat16, kind="ExternalInput").ap()
    wT = nc.dram_tensor("wT", [128, 2, K * C], dt.bfloat16, kind="ExternalInput").ap()
    idx = nc.dram_tensor("idx", [128, IDXC], dt.int16, kind="ExternalInput").ap()
    pmask = nc.dram_tensor("pmask", [128, 1], dt.float32, kind="ExternalInput").ap()
    maskI = nc.dram_tensor("maskI", [128, PB], dt.float32, kind="ExternalInput").ap()
    out = nc.dram_tensor("out", [1, 1], dt.float32, kind="ExternalOutput").ap()

    with TileContext(nc) as tc:
        nc.gpsimd.load_library(library_config.mlp)
        with (
            tc.tile_pool(name="const", bufs=1) as cp,
            tc.tile_pool(name="gat", bufs=14) as gp,
            tc.tile_pool(name="scr", bufs=4) as sp,
            tc.tile_pool(name="acc", bufs=1) as ap_,
        ):
            # --- constant / weight loads -------------------------------------
            latT_t = cp.tile([128, 2, T], dt.bfloat16)
            nc.sync.dma_start(latT_t[:], latT[:])
            wT_t = cp.tile([128, 2, K * C], dt.bfloat16)
            nc.sync.dma_start(wT_t[:], wT[:])
            pmask_t = cp.tile([128, 1], dt.float32)
            nc.sync.dma_start(pmask_t[:], pmask[:])
            maskI_t = cp.tile([128, PB], dt.float32)
            nc.sync.dma_start(maskI_t[:], maskI[:])
            negshift = cp.tile([128, 1], dt.float32)
            nc.vector.memset(negshift[:], -SHIFT)
            idx_t = cp.tile([128, IDXC], dt.int16)
            nc.sync.dma_start(idx_t[:], idx[:])

            # AzT store, tile-major: azsb[p, u*256 + h*128 + q*32 + k]
            # = Az[k, h*128+p, t=4u+q] (bf16). Each (tile u, c-half h) owns a
            # contiguous 128-col slab of 4 position sub-slabs (12 real k + 20
            # zero pad cols so pad output partitions produce zero scores).
            azsb = ap_.tile([128, TPAD * 64], dt.bfloat16)
            azsb6 = azsb.rearrange("p (u hh q j) -> p u hh q j", hh=2, q=4, j=32)
            # only the pad cols need zeroing; the Az copies write every k<12
            # col (including t>=500 tiles), so this runs concurrently
            for h_ in range(2):
                nc.vector.memset(azsb6[:, :, h_, :, K:32], 0.0)

            tot_all = ap_.tile([128, TPAD // 4], dt.float32)
            num_all = ap_.tile([128, TPAD // 4], dt.float32)

            # --- Az phase: AzT[kc, t] = sum_c' W[kc, c'] latent[n, t, c'] ----
            with tc.tile_pool(name="az_ps", bufs=2, space="PSUM") as azps:
                for b in range(2 * K):  # kc tile: kc = b*128 + p
                    k_, h_ = b // 2, b % 2
                    pa = azps.tile([128, T], dt.float32, name="pa")
                    for hp in range(2):  # contraction half
                        nc.tensor.matmul(
                            pa[:, :],
                            lhsT=wT_t[:, hp, b * 128 : (b + 1) * 128],
                            rhs=latT_t[:, hp, :],
                            start=(hp == 0),
                            stop=(hp == 1),
                        )
                    nc.scalar.copy(out=azsb6[:, :, h_, :, k_], in_=pa[:, :])

            # --- negative gathers (t < 500 only) -----------------------------
            nidx_regs = {
                PPG * M: nc.gpsimd.to_reg(PPG * M),
                LAST_P * M: nc.gpsimd.to_reg(LAST_P * M),
            }
            ng_tiles = []
            prev_gather = None
            _ngath = NGATH
            for g in range(NGATH):
                if g >= _ngath:
                    ng_tiles.append(None)
                    continue
                npos = PPG if g < NG_FULL else LAST_P
                nidx = npos * M
                if g < NG_FULL:
                    g_t = gp.tile([128, 2, PPG * M], dt.bfloat16, tag="ng", name="ng")
                else:
                    g_t = gp.tile([128, 2, nidx], dt.bfloat16, tag="ng_last", name="ngl")
                ics = g * (PPG * M // 16)
                gi = nc.gpsimd.dma_gather(
                    g_t[:],
                    lat_all[:],
                    idx_t[:, ics : ics + nidx // 16],
                    nidx,
                    nidx_regs[nidx],
                    C,
                    transpose=True,
                    queue_num=g % NQ,
                )
                # Pin gather scheduling order: the Tile DMASW-lane round-robin
                # must stay in lockstep with queue_num (a DMASW sem is locked
                # to one SWDGE queue).
                if prev_gather is not None:
                    add_dep_helper(gi.ins, prev_gather.ins, sync=False, reason="gather order")
                prev_gather = gi
                ng_tiles.append(g_t)

            # --- score megatiles ---------------------------------------------
            _nmega = TPAD // 16
            with tc.tile_pool(name="sc_ps", bufs=2, space="PSUM") as scps:
                for mega in range(_nmega):
                    P = scps.tile([128, 4, 512], dt.float32, name="P")
                    exp_i = None
                    for s in range(4):  # bank = one 4-position score tile
                        tile_idx = mega * 4 + s
                        t0 = tile_idx * 4
                        # positive cols: one shared 15-col window for all 4
                        # positions of the tile (diag shifts by col group q)
                        pt = min(1 + t0, T - PB)  # clamp pads in-bounds
                        if "nomm" in _ABL:
                            continue
                        # batched positives: strided 128-col lhsT covers the 4
                        # positions' weight slabs -> one group over all rows
                        pos_close = None
                        for h in range(2):
                            slab4 = azsb[:, tile_idx * 256 + h * 128 : tile_idx * 256 + (h + 1) * 128]
                            pos_close = nc.tensor.matmul(
                                P[:, s, 0:PB],
                                lhsT=slab4,
                                rhs=latT_t[:, h, pt : pt + PB],
                                start=(h == 0),
                                stop=(h == 1),
                            )
                        for q in range(4):  # column group: own 2-MM group
                            t = t0 + q
                            for h in range(2):
                                if t < Tp:
                                    g, pl = t // PPG, t % PPG
                                    nrhs = ng_tiles[g][:, h, M * pl : M * (pl + 1)]
                                else:  # dummy position: zero weights, any rhs
                                    nrhs = latT_t[:, h, 0:M]
                                slab = azsb[
                                    :,
                                    tile_idx * 256 + h * 128 + q * 32 : tile_idx * 256 + h * 128 + q * 32 + 32,
                                ]
                                mm = nc.tensor.matmul(
                                    P[32 * q : 32 * q + 32, s, PB : PB + M],
                                    lhsT=slab,
                                    rhs=nrhs,
                                    start=(h == 0),
                                    stop=(h == 1),
                                    tile_position=(0, 32 * q),
                                )
                                if h == 0:
                                    # the neg group's start clears the bank's
                                    # has_written rows: order it after the
                                    # positive group closes
                                    add_dep_helper(mm.ins, pos_close.ins, sync=False, reason="pos first")
                    if "notail" in _ABL:
                        continue
                    # tot[t,k] = sum_m exp(score-50): one exp over all 4 banks
                    E4 = sp.tile([128, 4, M], dt.float32, tag="exp", name="exp_o")
                    exp_i = nc.scalar.activation(
                        out=E4[:],
                        in_=P[:, :, PB : PB + M],
                        func=mybir.ActivationFunctionType.Exp,
                        bias=negshift[:],
                        scale=1.0,
                    )
                    nc.vector.tensor_reduce(
                        tot_all[:, mega * 4 : mega * 4 + 4],
                        E4[:],
                        axis=mybir.AxisListType.X,
                        op=mybir.AluOpType.add,
                    )
                    # num[t,k] -> num_all (shifted diagonal of the pos block),
                    # after the exp so every bank group is closed
                    for s in range(4):
                        tile_idx = mega * 4 + s
                        scr = sp.tile([128, PB], dt.float32, tag="ttr", name="ttr_o")
                        mul_i = nc.vector.tensor_mul(scr[:], P[:, s, 0:PB], maskI_t[:])
                        add_dep_helper(mul_i.ins, exp_i.ins, sync=True, reason="groups closed")
                        nc.vector.tensor_reduce(
                            num_all[:, tile_idx : tile_idx + 1],
                            scr[:],
                            axis=mybir.AxisListType.X,
                            op=mybir.AluOpType.add,
                        )
                    # fold in the positive term: tot += exp(num - 50)
                    en_t = sp.tile([128, 4], dt.float32, tag="en", name="en_t")
                    nc.scalar.activation(
                        out=en_t[:],
                        in_=num_all[:, mega * 4 : mega * 4 + 4],
                        func=mybir.ActivationFunctionType.Exp,
                        bias=negshift[:],
                        scale=1.0,
                    )
                    nc.vector.tensor_add(
                        tot_all[:, mega * 4 : mega * 4 + 4],
                        tot_all[:, mega * 4 : mega * 4 + 4],
                        en_t[:],
                    )

            # --- final reduction --------------------------------------------
            if "nofin" in _ABL:
                dummy = ap_.tile([1, 1], dt.float32)
                nc.vector.memset(dummy[:], 0.0)
                nc.sync.dma_start(out[:], dummy[:])
            else:
                NV = Tp // 4  # 125 valid score tiles
                # ln(tot * 2^-32) keeps the ACT-ln input within its 2^64 valid
                # range for extreme scores; +32*ln2 is restored on the host.
                Lt = ap_.tile([128, NV], dt.float32)
                nc.scalar.activation(
                    out=Lt[:],
                    in_=tot_all[:, :NV],
                    func=mybir.ActivationFunctionType.Ln,
                    scale=float(2.0**-32),
                )
                Dt = ap_.tile([128, NV], dt.float32)
                rs = ap_.tile([128, 1], dt.float32)
                nc.vector.tensor_sub(Dt[:], Lt[:], num_all[:, :NV])
                nc.vector.tensor_reduce(
                    rs[:],
                    Dt[:],
                    axis=mybir.AxisListType.X,
                    op=mybir.AluOpType.add,
                )
                with tc.tile_pool(name="f_ps", bufs=1, space="PSUM") as fps:
                    psf = fps.tile([1, 1], dt.float32)
                    nc.tensor.matmul(psf[:], lhsT=rs[:], rhs=pmask_t[:])
                    outsb = ap_.tile([1, 1], dt.float32)
                    nc.scalar.copy(out=outsb[:], in_=psf[:])
                    nc.sync.dma_start(out[:], outsb[:])

    nc.compile()
    return nc


def prep_inputs(latent, W, samps):
    """Host-side sharding + layout marshalling. Returns per-core input maps."""
    latent = np.asarray(latent, dtype=np.float32)
    W = np.asarray(W, dtype=np.float32)
    samps = np.asarray(samps).astype(np.int64).reshape(N, Tp, M)

    lat_all = latent.reshape(N * T, C).astype(BF16)
    wT = np.ascontiguousarray(
        W.T.astype(BF16).reshape(2, 128, K * C).transpose(1, 0, 2)
    )
    pmask = ((np.arange(128) % 32) < K).astype(np.float32).reshape(128, 1)
    q_arr, k_arr = np.arange(128) // 32, np.arange(128) % 32
    maskI = (
        (np.arange(15)[None, :] == (q_arr + k_arr)[:, None]) & (k_arr < K)[:, None]
    ).astype(np.float32)

    in_maps = []
    for n in range(N):
        latT = np.ascontiguousarray(
            latent[n].T.astype(BF16).reshape(2, 128, T).transpose(1, 0, 2)
        )
        # negative gather indices, wrapped: idx[p, g*56 + s] = flat_g[s*16 + p%16]
        flat = samps[n].reshape(Tp * M).astype(np.int16)  # position-major
        wrapped = flat.reshape(IDXC, 16).T  # [16, IDXC]
        idx = np.ascontiguousarray(np.tile(wrapped, (8, 1)))
        in_maps.append(
            {
                "lat_all": lat_all,
                "latT": latT,
                "wT": wT,
                "idx": idx,
                "pmask": pmask,
                "maskI": maskI,
            }
        )
    return in_maps


_NC_CACHE = None


def kernel(latent, W, samps):
    global _NC_CACHE
    from concourse import bass_utils

    if _NC_CACHE is None:
        _NC_CACHE = build_bass()
    nc = _NC_CACHE
    in_maps = prep_inputs(latent, W, samps)
    res = bass_utils.run_bass_kernel_spmd(nc, in_maps, core_ids=list(range(N)))
    partial = sum(float(r["out"][0, 0]) for r in res.results)
    import math

    return np.float32(partial / DENOM + SHIFT + 32.0 * math.log(2.0))
